# revision 5
# baseline (speedup 1.0000x reference)
"""Banded-DTW 1-NN (KnnDtw) Trainium2 Bass kernel.

Algorithm
---------
Reference computes, per (query q, fit row f), a Sakoe-Chiba banded DTW
(w=10) over length-256 sequences and returns fit_labels[argmin_f dm[q,f]].

Device mapping: in band ("buffer") coordinates, row i keeps 21 cells
c in [0,20] with j = i-10+c.  The update is
    a[c]   = min(prev[c], prev[c+1])
    row[c] = min(a[c], row[c-1]) + |samples[q,i] - fit[f, i-10+c]|
which is exactly one `tensor_tensor_scan` (op0=min, op1=add) per DTW step,
plus one `tensor_tensor` min and one scalar-engine Abs per step.
Out-of-range fit positions are padded with LARGE so band edges fall out
automatically; a -LARGE/+2*LARGE guard element between per-pair segments
resets the scan carry, letting one scan instruction process 32 independent
(q, f) pairs per partition.

Sharding: queries are split across 8 cores (16 each).  Per core the
4096 (q,f) pairs sit on 128 partitions x 32 segments.  Partition
p = q_local*8 + f_hi, segment s -> f = f_hi*32 + s, so the per-partition
activation bias (-samples[q, i]) is constant per partition.

The device returns dm (cost[255,255] per pair); the host does the final
argmin + label gather (trivial, exact).
"""

import numpy as np

import concourse.bass as bass
import concourse.bacc as bacc
import concourse.mybir as mybir
from concourse.tile import TileContext
from concourse import bass_utils

# Problem shapes (hardcoded per harness contract)
NQ, M = 128, 256      # samples
NF, N = 256, 256      # fit_data
NCORES = 8
QPC = NQ // NCORES    # 16 queries per core
CELLS = 21            # band cells per row, c in [0,20], j = i-10+c
SEG = CELLS + 1       # +1 guard element that resets the scan carry
NSEG = 32             # segments (f_lo values) per partition
FD = NSEG * SEG       # 704 scan elements per partition
PAD = 16              # fit row padding on each side
PADF = N + 2 * PAD    # 288
LARGE = np.float32(1e15)
F32 = mybir.dt.float32

_CACHE: dict = {}


def _build_nc() -> bass.Bass:
    nc = bacc.Bacc(
        "TRN2", target_bir_lowering=False, debug=False, num_devices=NCORES
    )

    fit_in = nc.dram_tensor("fit_rep", [128, NSEG * PADF], F32, kind="ExternalInput")
    nsamp_in = nc.dram_tensor("neg_samp", [128, M], F32, kind="ExternalInput")
    row0_in = nc.dram_tensor("row0", [128, FD], F32, kind="ExternalInput")
    atmpl_in = nc.dram_tensor("a_tmpl", [128, FD], F32, kind="ExternalInput")
    dtmpl_in = nc.dram_tensor("d_tmpl", [128, FD], F32, kind="ExternalInput")
    dm_out = nc.dram_tensor("dm_out", [128, NSEG], F32, kind="ExternalOutput")

    amin = mybir.AluOpType.min
    aadd = mybir.AluOpType.add
    fabs = mybir.ActivationFunctionType.Abs

    with TileContext(nc) as tc:
        with tc.tile_pool(name="main", bufs=1) as pool:
            fit_sb = pool.tile([128, NSEG * PADF], F32)
            nsamp = pool.tile([128, M], F32)
            row_a = pool.tile([128, FD], F32)
            row_b = pool.tile([128, FD], F32)
            a_arr = pool.tile([128, FD], F32)
            d_a = pool.tile([128, FD], F32)
            d_b = pool.tile([128, FD], F32)
            dmc = pool.tile([128, NSEG], F32)

            nc.sync.dma_start(out=fit_sb[:], in_=fit_in[:, :])
            nc.sync.dma_start(out=nsamp[:], in_=nsamp_in[:, :])
            nc.sync.dma_start(out=row_a[:], in_=row0_in[:, :])
            nc.sync.dma_start(out=a_arr[:], in_=atmpl_in[:, :])
            nc.sync.dma_start(out=d_a[:], in_=dtmpl_in[:, :])
            nc.sync.dma_start(out=d_b[:], in_=dtmpl_in[:, :])

            fit3 = fit_sb.rearrange("p (s c) -> p s c", c=PADF)
            a3 = a_arr.rearrange("p (s c) -> p s c", c=SEG)
            rows = [row_a, row_b]
            rows3 = [r.rearrange("p (s c) -> p s c", c=SEG) for r in rows]
            ds = [d_a, d_b]
            ds3 = [d.rearrange("p (s c) -> p s c", c=SEG) for d in ds]

            for i in range(1, M):
                rin3 = rows3[(i - 1) % 2]
                rout = rows[i % 2]
                dt = ds[i % 2]
                dt3 = ds3[i % 2]
                # d[c] = |fit[f, i-10+c] - samples[q, i]|, c in [0,20)
                nc.scalar.activation(
                    out=dt3[:, :, 1:21],
                    in_=fit3[:, :, i + PAD - 10 : i + PAD + 10],
                    func=fabs,
                    bias=nsamp[:, i : i + 1],
                    scale=1.0,
                )
                # a[c] = min(prev[c], prev[c+1])
                nc.vector.tensor_tensor(
                    out=a3[:, :, 1:21],
                    in0=rin3[:, :, 1:21],
                    in1=rin3[:, :, 2:22],
                    op=amin,
                )
                # row[c] = min(a[c], carry) + d[c]  (segmented via guards)
                nc.vector.tensor_tensor_scan(
                    out=rout[:, :],
                    data0=a_arr[:, :],
                    data1=dt[:, :],
                    initial=float(LARGE),
                    op0=amin,
                    op1=aadd,
                )

            # dm = cost[255,255] = final row cell c=10 (offset 11 per segment)
            rf3 = rows3[(M - 1) % 2]
            dmc3 = dmc.rearrange("p (s o) -> p s o", o=1)
            nc.vector.tensor_copy(out=dmc3[:, :, 0:1], in_=rf3[:, :, 11:12])
            nc.sync.dma_start(out=dm_out[:, :], in_=dmc[:])

    nc.compile()
    return nc


def _host_inputs(samples: np.ndarray, fit: np.ndarray):
    """Per-core in_maps for run_bass_kernel_spmd."""
    pidx = np.arange(128)
    fidx = (pidx % NCORES)[:, None] * NSEG + np.arange(NSEG)[None, :]  # [128,32]

    fit_pad = np.full((NF, PADF), LARGE, np.float32)
    fit_pad[:, PAD : PAD + N] = fit
    fit_rep = np.ascontiguousarray(fit_pad[fidx].reshape(128, NSEG * PADF))

    a_tmpl = np.full((128, NSEG, SEG), LARGE, np.float32)
    a_tmpl[:, :, 0] = -LARGE
    a_tmpl[:, :, 21] = LARGE
    a_tmpl = a_tmpl.reshape(128, FD)

    d_tmpl = np.full((128, NSEG, SEG), LARGE, np.float32)
    d_tmpl[:, :, 0] = 2 * LARGE
    d_tmpl[:, :, 21] = 2 * LARGE
    d_tmpl = d_tmpl.reshape(128, FD)

    in_maps = []
    for core in range(NCORES):
        qidx = core * QPC + pidx // NCORES  # [128]
        neg_samp = np.ascontiguousarray(-samples[qidx])

        row0 = np.full((128, NSEG, SEG), LARGE, np.float32)
        d0 = np.abs(samples[qidx, 0][:, None, None] - fit[fidx][:, :, 0:11])
        row0[:, :, 11:22] = np.cumsum(d0.astype(np.float32), axis=-1, dtype=np.float32)
        row0 = row0.reshape(128, FD)

        in_maps.append(
            {
                "fit_rep": fit_rep,
                "neg_samp": neg_samp,
                "row0": np.ascontiguousarray(row0),
                "a_tmpl": a_tmpl,
                "d_tmpl": d_tmpl,
            }
        )
    return in_maps


def _assemble_dm(results) -> np.ndarray:
    dm = np.empty((NQ, NF), np.float32)
    for core, res in enumerate(results):
        arr = np.asarray(res["dm_out"], np.float32).reshape(QPC, NCORES, NSEG)
        dm[core * QPC : (core + 1) * QPC] = arr.reshape(QPC, NF)
    return dm


def run_device(samples, fit, **spmd_kwargs):
    """Compile (cached) + run on 8 cores; returns (dm [128,256], BassKernelResults)."""
    if "nc" not in _CACHE:
        _CACHE["nc"] = _build_nc()
    nc = _CACHE["nc"]
    in_maps = _host_inputs(samples, fit)
    res = bass_utils.run_bass_kernel_spmd(
        nc, in_maps, core_ids=list(range(NCORES)), **spmd_kwargs
    )
    return _assemble_dm(res.results), res


def kernel(samples, fit_data, fit_labels):
    samples = np.ascontiguousarray(np.asarray(samples), dtype=np.float32)
    fit = np.ascontiguousarray(np.asarray(fit_data), dtype=np.float32)
    labels = np.asarray(fit_labels)
    dm, _ = run_device(samples, fit)
    knn = np.argmin(dm, axis=1)
    return labels[knn]
